# revision 17
# baseline (speedup 1.0000x reference)
"""Trainium2 Bass kernel for nn_GCIQEValue (MLP + IQE head), 8-core data parallel.

Math (validated vs reference):
  phi(x) = LN-MLP: 3x [matmul+bias -> tanh-gelu -> LayerNorm(affine folded into
  next W on host)] then final matmul+bias.
  IQE per row, per 32-dim component c with x = phi_s[c], y = phi_g[c]:
    y' = max(x, y); u = sort(x), v = sort(y')
    comp_c = sum(v) - u_0 - sum_{i>=1} max(u_i, v_{i-1})
  out = sig(alpha) * mean_c(comp) + (1 - sig(alpha)) * max_c(comp)

v2 design:
  - All matmuls bf16 (PSUM accumulates fp32); tolerance is 2e-2, measured ~4e-3.
  - LayerNorm is never materialized: with m = mean(g), rho = rsqrt(var+eps),
    q = 1/rho, the next layer computes
      z_acc = g @ W~ + m*(-colsum W~) + q*b~        (K=2 rank-1 rides the PE)
      z     = rho * z_acc                           (rides ACT gelu `scale`)
    so the only LN cost is the ACT accum stats + a short fp32 Newton-rsqrt
    chain on DVE + one tiny [128,4] PE transpose of (m,q) per layer.
  - All activation transposes for the next matmul are DMA xbar transposes
    (SBUF->SBUF, and DRAM->SBUF for the input layer) - no PE transposes, no
    PSUM->SBUF copies on ACT/Pool.
  - IQE sort: phi_s / y' are interleaved per 32-bit word (even bf16 lane = x
    network, odd = y network), so every bitonic pass is a pair of step-1
    2-byte tensor_tensor min/max ops -> DVE 2x packed mode for all 15 passes.
    The coupling max(u_i, v_{i-1}) runs in place on the even lanes; comp then
    equals (sum of odd lanes) - (sum of even lanes) per component.
"""

import numpy as np

B = 131072
OBS = 64
H = 512
NCOMP = 16
DPC = 32
NCORES = 8
P = 128
LN_EPS = 1e-6

_CACHE = {}

# bitonic schedule for 32-wide ascending sort: 15 passes (word-level)
_SCHED = [("pair", 0, 0)]
for _L in (4, 8, 16, 32):
    _SCHED.append(("flip", _L, 0))
    _d = _L // 4
    while _d >= 1:
        _SCHED.append(("shift", _L, _d))
        _d //= 2


# ---------------------------------------------------------------- device kernel
def build_nc(rows_per_core=B // NCORES, unroll=8, gelu="hw", repeats=1,
             stage_bufs=8, mlp_bufs=6, psum_bufs=5, split_pass=8,
             n_passes=15, newton_iters=1, red_pool=1, couple_pool=0,
             stat_copy="act", hints=False):
    """Build the Bass (Bacc) module for one core processing rows_per_core rows."""
    import concourse.bass as bass
    import concourse.mybir as mybir
    import concourse.tile as tile
    from concourse import bacc
    from concourse.masks import make_identity

    fp32 = mybir.dt.float32
    i32 = mybir.dt.int32
    mdt = mybir.dt.bfloat16
    AT = mybir.ActivationFunctionType
    OP = mybir.AluOpType

    nt = rows_per_core // P
    assert rows_per_core % P == 0

    nc = bacc.Bacc("TRN2", target_bir_lowering=False, debug=False)

    obs = nc.declare_dram_parameter("observations", [rows_per_core, OBS], mdt,
                                    isOutput=False)
    gls = nc.declare_dram_parameter("goals", [rows_per_core, OBS], mdt,
                                    isOutput=False)
    w0d = nc.declare_dram_parameter("w0", [OBS, H], mdt, isOutput=False)
    w1d = nc.declare_dram_parameter("w1", [H, H], mdt, isOutput=False)
    w2d = nc.declare_dram_parameter("w2", [H, H], mdt, isOutput=False)
    w3d = nc.declare_dram_parameter("w3", [H, H], mdt, isOutput=False)
    b0d = nc.declare_dram_parameter("b0r", [1, H], mdt, isOutput=False)
    cbd = nc.declare_dram_parameter("cb", [2, 3, H], mdt, isOutput=False)
    avd = nc.declare_dram_parameter("avec", [P, 2], fp32, isOutput=False)
    out = nc.declare_dram_parameter("out", [rows_per_core], fp32, isOutput=True)

    obs_v = obs[:].rearrange("(n p) f -> n p f", p=P)
    gls_v = gls[:].rearrange("(n p) f -> n p f", p=P)
    out_v = out[:].rearrange("(n p) -> n p", p=P)

    gelu_f = AT.Gelu_apprx_tanh if gelu == "hw" else AT.Identity

    with tile.TileContext(nc) as tc:
        with (
            tc.tile_pool(name="const", bufs=1) as cpool,
            tc.tile_pool(name="mlp", bufs=mlp_bufs) as mp,
            tc.tile_pool(name="srt", bufs=mlp_bufs) as sp,
            tc.tile_pool(name="pipe", bufs=1) as pipe_pool,
            tc.tile_pool(name="ps", bufs=psum_bufs, space="PSUM") as pp,
            tc.tile_pool(name="pss", bufs=2, space="PSUM") as pps,
        ):
            # ---- constants
            w0 = cpool.tile([OBS, H], mdt)
            nc.sync.dma_start(out=w0, in_=w0d[:])
            wl = []
            for wd, nm in ((w1d, "w1"), (w2d, "w2"), (w3d, "w3")):
                t = cpool.tile([P, 4, H], mdt, tag=nm)
                nc.sync.dma_start(out=t, in_=wd[:].rearrange("(c p) n -> p c n", p=P))
                wl.append(t)
            b0c = cpool.tile([1, H], mdt)
            nc.sync.dma_start(out=b0c, in_=b0d[:])
            cbt = cpool.tile([2, 3, H], mdt)
            nc.sync.dma_start(out=cbt, in_=cbd[:])
            avec = cpool.tile([P, 2], fp32)
            nc.sync.dma_start(out=avec, in_=avd[:])
            ident = cpool.tile([P, P], mdt)
            make_identity(nc, ident)
            ones = cpool.tile([1, P], mdt)
            nc.vector.memset(ones, 1.0)

            def mm_layer(li, tT, st2):
                """tT: transposed activations ([OBS,P] for li=0 else [P,4,P]);
                st2: [2,P] view of (m, q) stats (None for li=0).
                Returns pz PSUM [P, H] = z_acc."""
                pz = pp.tile([P, H], fp32, tag="pz")
                if li == 0:
                    nc.tensor.matmul(pz, tT, w0, start=True, stop=False)
                    nc.tensor.matmul(pz, ones, b0c, start=False, stop=True)
                else:
                    for k in range(4):
                        nc.tensor.matmul(pz, tT[:, k, :], wl[li - 1][:, k, :],
                                         start=(k == 0), stop=False)
                    nc.tensor.matmul(pz, st2, cbt[:, li - 1, :], start=False,
                                     stop=True)
                return pz

            def gelu_stats(pz, g, sums2, rho1):
                """ACT: g = gelu(pz * rho1), accum -> sums2[:,0:1];
                Square(g) accum -> sums2[:,1:2]."""
                scale = rho1 if rho1 is not None else 1.0
                nc.scalar.activation(g, pz, gelu_f, scale=scale,
                                     accum_out=sums2[:, 0:1])
                gsq = mp.tile([P, H], mdt, tag="gsq")
                nc.scalar.activation(gsq, g, AT.Square, accum_out=sums2[:, 1:2])

            def ln_stats(sums, pipe, li):
                """sums [P,4] fp32 (s_o, sq_o, s_g, sq_g) -> (rho [P,2] fp32,
                stT [4,P] bf16 rows (m_o, q_o, m_g, q_g))."""
                mv = mp.tile([P, 4], fp32, tag="mv")
                nc.vector.tensor_scalar_mul(mv, sums, 1.0 / H)
                means = mv.rearrange("p (a b) -> p a b", b=2)[:, :, 0:1]
                msqs = mv.rearrange("p (a b) -> p a b", b=2)[:, :, 1:2]
                m2 = mp.tile([P, 2], fp32, tag="m2")
                nc.vector.tensor_tensor(out=m2, in0=means, in1=means, op=OP.mult)
                vt = mp.tile([P, 2], fp32, tag="vt")
                nc.vector.scalar_tensor_tensor(out=vt, in0=msqs, scalar=LN_EPS,
                                               in1=m2, op0=OP.add,
                                               op1=OP.subtract)
                rho = pipe.intermediate_tile([P, 2], fp32, name=f"rho{li}")
                ri = rho.bitcast(i32)
                nc.vector.tensor_scalar(out=ri, in0=vt.bitcast(i32), scalar1=1,
                                        scalar2=None, op0=OP.logical_shift_right)
                nc.vector.tensor_scalar(out=ri, in0=ri, scalar1=-1,
                                        scalar2=0x5F3759DF, op0=OP.mult,
                                        op1=OP.add)
                t1 = mp.tile([P, 2], fp32, tag="nt1")
                for _ in range(newton_iters):
                    nc.vector.tensor_tensor(out=t1, in0=vt, in1=rho, op=OP.mult)
                    nc.vector.tensor_tensor(out=t1, in0=t1, in1=rho, op=OP.mult)
                    nc.vector.tensor_scalar(out=t1, in0=t1, scalar1=-0.5,
                                            scalar2=1.5, op0=OP.mult, op1=OP.add)
                    nc.vector.tensor_tensor(out=rho, in0=rho, in1=t1, op=OP.mult)
                # q = vt * rho overwrites the meansq slots -> mv = (m, q, m, q)
                nc.vector.tensor_tensor(out=msqs, in0=vt, in1=rho, op=OP.mult)
                asm = mp.tile([P, 4], mdt, tag="asm")
                nc.vector.tensor_copy(asm, mv)
                stP = pps.tile([2, 2, P], mdt, tag="stP")
                nc.tensor.transpose(stP[:, 0, :], asm[:, 0:2], ident)
                nc.tensor.transpose(stP[:, 1, :], asm[:, 2:4], ident)
                stTo = pipe.intermediate_tile([2, P], mdt, name=f"stTo{li}")
                stTg = pipe.intermediate_tile([2, P], mdt, name=f"stTg{li}")
                cp = nc.scalar.copy if stat_copy == "act" else nc.vector.tensor_copy
                cp(stTo, stP[:, 0, :])
                cp(stTg, stP[:, 1, :])
                return rho, stTo, stTg

            # ---------------- pipeline stages
            def st_load(pipe, iv):
                xT = pipe.intermediate_tile([OBS, P], mdt, name="xT")
                gT = pipe.intermediate_tile([OBS, P], mdt, name="gT")
                nc.sync.dma_start_transpose(xT, obs_v[iv])
                nc.sync.dma_start_transpose(gT, gls_v[iv])
                return (xT, gT)

            def mk_layer(li):
                def st(pipe, iv, prev):
                    if li == 0:
                        tTo, tTg = prev
                        sto = stg = None
                        rho = None
                    else:
                        tTo, tTg, sto, stg, rho = prev
                    sums = mp.tile([P, 4], fp32, tag="sums")
                    pzo = mm_layer(li, tTo, sto)
                    go = mp.tile([P, H], mdt, tag="g")
                    gelu_stats(pzo, go, sums.rearrange("p (a b) -> p a b", b=2)[:, 0, :],
                               rho[:, 0:1] if rho is not None else None)
                    pzg = mm_layer(li, tTg, stg)
                    gg = mp.tile([P, H], mdt, tag="g")
                    gelu_stats(pzg, gg, sums.rearrange("p (a b) -> p a b", b=2)[:, 1, :],
                               rho[:, 1:2] if rho is not None else None)
                    rho_n, stTo_n, stTg_n = ln_stats(sums, pipe, li)
                    tTo_n = pipe.intermediate_tile([P, 4, P], mdt, name=f"tT{li}o")
                    tTg_n = pipe.intermediate_tile([P, 4, P], mdt, name=f"tT{li}g")
                    nc.sync.dma_start_transpose(tTo_n, go)
                    nc.sync.dma_start_transpose(tTg_n, gg)
                    return (tTo_n, tTg_n, stTo_n, stTg_n, rho_n)
                return st

            def st_l3(pipe, iv, prev):
                tTo, tTg, sto, stg, rho = prev
                pzo = mm_layer(3, tTo, sto)
                pzg = mm_layer(3, tTg, stg)
                ib = pipe.intermediate_tile([P, 2 * H], mdt, name="ib")
                iv2 = ib.rearrange("p (j e) -> p j e", e=2)
                nc.scalar.activation(iv2[:, :, 0:1], pzo, AT.Identity,
                                     scale=rho[:, 0:1])
                nc.scalar.activation(iv2[:, :, 1:2], pzg, AT.Identity,
                                     scale=rho[:, 1:2])
                # odd lanes <- y' = max(phi_s, phi_g)
                nc.vector.tensor_tensor(out=iv2[:, :, 1:2], in0=iv2[:, :, 0:1],
                                        in1=iv2[:, :, 1:2], op=OP.max)
                return ib

            def emit_sort_pass(p_idx, cur, nxt):
                """Word-level bitonic pass p_idx: cur -> nxt ([P,1024] bf16).
                Words j in [0,32) per 16 blocks; element = 2*word + lane."""
                kind, L, d = _SCHED[p_idx]
                V = nc.vector
                if kind == "pair":
                    s = cur.rearrange("p (g e) -> p g e", e=4)
                    o = nxt.rearrange("p (g e) -> p g e", e=4)
                    V.tensor_tensor(out=o[:, :, 0:2], in0=s[:, :, 0:2],
                                    in1=s[:, :, 2:4], op=OP.min)
                    V.tensor_tensor(out=o[:, :, 2:4], in0=s[:, :, 0:2],
                                    in1=s[:, :, 2:4], op=OP.max)
                elif kind == "flip":
                    hw_ = L  # words per block half*2
                    s = cur.rearrange("p (b w e) -> p b w e", w=L, e=2)
                    o = nxt.rearrange("p (b w e) -> p b w e", w=L, e=2)
                    V.tensor_tensor(out=o[:, :, 0:L // 2, :],
                                    in0=s[:, :, 0:L // 2, :],
                                    in1=s[:, :, L - 1:L // 2 - 1:-1, :],
                                    op=OP.min)
                    V.tensor_tensor(out=o[:, :, L // 2:L, :],
                                    in0=s[:, :, L // 2:L, :],
                                    in1=s[:, :, L // 2 - 1::-1, :], op=OP.max)
                else:
                    s = cur.rearrange("p (c e) -> p c e", e=4 * d)
                    o = nxt.rearrange("p (c e) -> p c e", e=4 * d)
                    V.tensor_tensor(out=o[:, :, 0:2 * d], in0=s[:, :, 0:2 * d],
                                    in1=s[:, :, 2 * d:4 * d], op=OP.min)
                    V.tensor_tensor(out=o[:, :, 2 * d:4 * d],
                                    in0=s[:, :, 0:2 * d],
                                    in1=s[:, :, 2 * d:4 * d], op=OP.max)

            def st_sort_a(pipe, iv, prev):
                ib = prev
                bufA = pipe.intermediate_tile([P, 2 * H], mdt, name="bufA")
                bufB = pipe.intermediate_tile([P, 2 * H], mdt, name="bufB")
                emit_sort_pass(0, ib, bufA)
                cur, nxt = bufA, bufB
                for pidx in range(1, split_pass):
                    if pidx < n_passes:
                        emit_sort_pass(pidx, cur, nxt)
                    cur, nxt = nxt, cur
                return (bufA, bufB)

            def st_sort_b(pipe, iv, prev):
                bufA, bufB = prev
                cur, nxt = (bufB, bufA) if split_pass % 2 == 0 else (bufA, bufB)
                for pidx in range(split_pass, 15):
                    if pidx < n_passes:
                        emit_sort_pass(pidx, cur, nxt)
                    cur, nxt = nxt, cur
                fin = cur
                v4 = fin.rearrange("p (c j e) -> p c j e", j=DPC, e=2)
                # coupling: even[j] <- max(even[j], odd[j-1]) for j>=1, in place
                eng = nc.gpsimd if couple_pool else nc.vector
                eng.tensor_tensor(out=v4[:, :, 1:DPC, 0:1],
                                  in0=v4[:, :, 1:DPC, 0:1],
                                  in1=v4[:, :, 0:DPC - 1, 1:2], op=OP.max)
                # per-block lane sums: red[p, c, e] = sum_j fin[p, c, j, e]
                vr = fin.rearrange("p (c j e) -> p c e j", j=DPC, e=2)
                red = sp.tile([P, NCOMP, 2], fp32, tag="red")
                nc.vector.tensor_reduce(out=red, in_=vr, axis=mybir.AxisListType.X,
                                        op=OP.add)
                comp = sp.tile([P, NCOMP], fp32, tag="comp")
                nc.vector.tensor_tensor(out=comp, in0=red[:, :, 1:2],
                                        in1=red[:, :, 0:1], op=OP.subtract)
                cs = sp.tile([P, 1], fp32, tag="cs")
                nc.vector.tensor_reduce(out=cs, in_=comp,
                                        axis=mybir.AxisListType.X, op=OP.add)
                cm = sp.tile([P, 1], fp32, tag="cm")
                nc.vector.tensor_reduce(out=cm, in_=comp,
                                        axis=mybir.AxisListType.X, op=OP.max)
                res = sp.tile([P, 1], fp32, tag="res")
                nc.vector.tensor_scalar(out=res, in0=cs, scalar1=avec[:, 0:1],
                                        scalar2=None, op0=OP.mult)
                nc.vector.scalar_tensor_tensor(out=res, in0=cm,
                                               scalar=avec[:, 1:2], in1=res,
                                               op0=OP.mult, op1=OP.add)
                nc.sync.dma_start(out=out_v[iv], in_=res[:, 0:1])

            stages = [st_load, mk_layer(0), mk_layer(1), mk_layer(2), st_l3,
                      st_sort_a, st_sort_b]

            def run_pipe():
                he = (mybir.EngineType.PE, mybir.EngineType.DVE,
                      mybir.EngineType.Activation, mybir.EngineType.SP,
                      mybir.EngineType.Pool) if hints else ()
                tc.For_i_pipelined(stages, 0, nt, 1, pool=pipe_pool,
                                   unroll=unroll, staged_num_bufs=stage_bufs,
                                   hint_engines=he)

            if repeats == 1:
                run_pipe()
            else:
                with tc.For_i(0, repeats, 1):
                    run_pipe()

    nc.finalize()
    return nc


# ---------------------------------------------------------------- host wrapper
def _prep_host(inputs):
    """Fold LN affine params into the following layer's weights; build the
    rank-1 correction table cb and the alpha mix vector."""
    f32 = np.float32
    W0 = np.asarray(inputs["W0"], f32)
    b0 = np.asarray(inputs["b0"], f32)
    w, b = [W0], [b0]
    for i in (0, 1, 2):
        s = np.asarray(inputs[f"ln{i}_s"], f32)
        t = np.asarray(inputs[f"ln{i}_b"], f32)
        Wn = np.asarray(inputs[("W1", "W2", "W3")[i]], f32)
        bn = np.asarray(inputs[("b1", "b2", "b3")[i]], f32)
        w.append(s[:, None] * Wn)
        b.append(bn + t @ Wn)
    # cb[0, li-1] = -colsum(W~_li), cb[1, li-1] = b~_li  for li = 1..3
    cb = np.empty((2, 3, H), f32)
    for li in (1, 2, 3):
        cb[0, li - 1] = -w[li].sum(axis=0)
        cb[1, li - 1] = b[li]
    alpha = float(np.asarray(inputs["alpha"]))
    a = 1.0 / (1.0 + np.exp(-alpha))
    avec = np.empty((P, 2), f32)
    avec[:, 0] = a / NCOMP
    avec[:, 1] = 1.0 - a
    return w[0], w[1], w[2], w[3], b[0][None, :], cb, avec


def _probe_devices():
    """Poke every core with a tiny op; retries to shake off a stale
    NRT_EXEC_UNIT_UNRECOVERABLE state left by a previous process."""
    import jax
    import jax.numpy as jnp

    for attempt in range(3):
        try:
            for d in jax.devices()[:NCORES]:
                jnp.zeros((1,), jnp.float32, device=d).block_until_ready()
            return
        except Exception:
            if attempt == 2:
                raise


def run_on_device(inputs, rows_total=B, trace=False, repeats=1, **build_kw):
    """Shard, run on 8 cores, gather. Returns (out [rows_total], results obj)."""
    import ml_dtypes
    from concourse.bass_utils import run_bass_kernel_spmd

    _probe_devices()

    rows_core = rows_total // NCORES
    key = (rows_core, repeats, tuple(sorted(build_kw.items())))
    if key not in _CACHE:
        _CACHE[key] = build_nc(rows_core, repeats=repeats, **build_kw)
    nc = _CACHE[key]

    mnp = ml_dtypes.bfloat16
    w0, w1, w2, w3, b0r, cb, avec = _prep_host(inputs)
    w0, w1, w2, w3, b0r, cb = (x.astype(mnp) for x in (w0, w1, w2, w3, b0r, cb))
    ob = np.ascontiguousarray(
        np.asarray(inputs["observations"], np.float32)[:rows_total].astype(mnp))
    gl = np.ascontiguousarray(
        np.asarray(inputs["goals"], np.float32)[:rows_total].astype(mnp))
    in_maps = []
    for c in range(NCORES):
        sl = slice(c * rows_core, (c + 1) * rows_core)
        in_maps.append({
            "observations": ob[sl], "goals": gl[sl],
            "w0": w0, "w1": w1, "w2": w2, "w3": w3, "b0r": b0r, "cb": cb,
            "avec": avec,
        })
    r = run_bass_kernel_spmd(nc, in_maps, list(range(NCORES)), trace=trace)
    outp = np.concatenate([r.results[c]["out"] for c in range(NCORES)])
    return outp, r


def kernel(**inputs):
    out, _ = run_on_device(inputs)
    return out.astype(np.float32)
